# revision 40
# baseline (speedup 1.0000x reference)
"""ConvShiftLayer TRN2 kernel: a = tanh(x@W+b); z = (a>0); z_conv = shift-conv(z).

Math: z_conv[t, o] = sum_{d=0..7} z[t+4-d, (o+d) % 1024]  (zero outside seq range)
Factored: A[p, v] = sum_{d0=0..3} z[v-d0, (p+d0)%F]; C[o, u] = A[o, u+7] + A[(o+4)%F, u+3].

Sharding: 8 cores = (batch 4) x (seq halves 2); each core computes a 512-step seq
slice + halo (3 left / 4 right) from scratch. On-chip layout: features on
partitions (8 tiles of 128), seq on the free axis (520 cols: 3 halo + 512 owned
+ 4 halo + 1 pad).

Main matmul: 3-term bf16 split (xh@wh + xl@wh + xh@wl) accumulated in fp32 psum
(error ~2^-16, exact enough for the z threshold). Bias enters through the
activation instruction's per-partition bias operand; halo columns outside the
sequence get pre-activation 0 -> z = (tanh(b) > 0), which is wrong only for
t<0 / t>=1024 when b != 0, and the host fixes those 7 boundary z_conv columns
from the (exact) device z.

Conv: banded shift-matrix matmuls in bf16 (exact on 0/1 data) accumulated in
psum. Stage A applies (I + Q + Q^2 + Q^3), stage B (I + Q^4); Q's feature shift
is the lhsT matrix (in-tile band + cross-tile wrap into tile f+1), its time
shift a free-axis rhs column offset. Stage B runs interleaved with the main
matmuls, borrowing the stage-A psum slot its gate proves free. No DRAM round
trips.

DMA strategy (each DMA instruction costs ~620ns to issue and one queue moves
~22GB/s): input issue is spread over four engine queues (sync: bias + xh-even +
weight quarters, chained so per-quarter gates are race-free; gpsimd: shift
matrices + xh-odd; scalar: xl; vector: first wl quarter). Producer engines
issue their own outputs (scalar: a; gpsimd: z; sync drains c as copies land).
Outputs are bf16 (a ~0.2% rounding; z, z_conv exact); host converts to f32.
"""
import numpy as np
import ml_dtypes
from contextlib import ExitStack

import concourse.bass as bass
import concourse.mybir as mybir
from concourse.bass_utils import run_bass_kernel_spmd

F_DIM = 1024
IN_DIM = 768
SEQ = 1024
BATCH = 4
T = 520          # 3 halo + 512 owned + 4 halo + 1 pad
OWN = 512
NK = 6           # K tiles of 128 over 768
NF = 8           # feature tiles of 128

f32 = mybir.dt.float32
bf16 = mybir.dt.bfloat16
bf16_np = ml_dtypes.bfloat16

LAST_RESULTS = None  # BassKernelResults of the most recent run (for test.py)


def _build_module():
    nc = bass.Bass()
    xh_in = nc.declare_dram_parameter("xh", [NK * 128, T], bf16, isOutput=False)
    xl_in = nc.declare_dram_parameter("xl", [NK * 128, T], bf16, isOutput=False)
    wh_in = nc.declare_dram_parameter("wh", [128, NF * NK * 128], bf16, isOutput=False)
    wl_in = nc.declare_dram_parameter("wl", [128, NF * NK * 128], bf16, isOutput=False)
    bias_in = nc.declare_dram_parameter("biasv", [128, NF], f32, isOutput=False)
    sh_in = nc.declare_dram_parameter("sh", [128, 9 * 128], bf16, isOutput=False)
    at_out = nc.declare_dram_parameter("at", [F_DIM, OWN], bf16, isOutput=True)
    zt_out = nc.declare_dram_parameter("zt", [F_DIM, OWN], bf16, isOutput=True)
    ct_out = nc.declare_dram_parameter("ct", [F_DIM, OWN], bf16, isOutput=True)

    ctx = ExitStack()
    with ctx:
        xh = [ctx.enter_context(nc.sbuf_tensor(f"xhs{k}", [128, T], bf16)) for k in range(NK)]
        xl = [ctx.enter_context(nc.sbuf_tensor(f"xls{k}", [128, T], bf16)) for k in range(NK)]
        wh_s = ctx.enter_context(nc.sbuf_tensor("whs", [128, NF * NK * 128], bf16))
        wl_s = ctx.enter_context(nc.sbuf_tensor("wls", [128, NF * NK * 128], bf16))
        bias_s = ctx.enter_context(nc.sbuf_tensor("biasvs", [128, NF], f32))
        sh = ctx.enter_context(nc.sbuf_tensor("shs", [128, 9 * 128], bf16))
        a_s = [ctx.enter_context(nc.sbuf_tensor(f"a{f}", [128, T], bf16)) for f in range(NF)]
        z_s = [ctx.enter_context(nc.sbuf_tensor(f"z{f}", [128, T], bf16)) for f in range(NF)]
        A_s = [ctx.enter_context(nc.sbuf_tensor(f"A{f}", [128, 516], bf16)) for f in range(NF)]
        cc = [ctx.enter_context(nc.sbuf_tensor(f"cc{f}", [128, OWN], bf16)) for f in range(NF)]
        # psum: 4 banks main matmul (pingpong pairs of 260+260), 4 banks conv
        # (stage A pingpong pairs of 258+258; stage B borrows the slot its
        # sAcp gate proves free).
        mA = [ctx.enter_context(nc.psum_tensor(f"mA{i}", [128, 260], f32)) for i in range(2)]
        mB = [ctx.enter_context(nc.psum_tensor(f"mB{i}", [128, 260], f32)) for i in range(2)]
        pA = [ctx.enter_context(nc.psum_tensor(f"pA{i}", [128, 258], f32)) for i in range(4)]

        dxh = ctx.enter_context(nc.semaphore("dxh"))
        dxg = ctx.enter_context(nc.semaphore("dxg"))
        dsh = ctx.enter_context(nc.semaphore("dsh"))
        dxl = ctx.enter_context(nc.semaphore("dxl"))
        dw0 = ctx.enter_context(nc.semaphore("dw0"))
        dwq = ctx.enter_context(nc.semaphore("dwq"))
        smm = ctx.enter_context(nc.semaphore("smm"))
        sza = ctx.enter_context(nc.semaphore("sza"))
        szz = ctx.enter_context(nc.semaphore("szz"))
        sAmm = ctx.enter_context(nc.semaphore("sAmm"))
        sAcp = ctx.enter_context(nc.semaphore("sAcp"))
        sBmm = ctx.enter_context(nc.semaphore("sBmm"))
        sCcpS = ctx.enter_context(nc.semaphore("sCcpS"))
        sCcpV = ctx.enter_context(nc.semaphore("sCcpV"))
        dout = ctx.enter_context(nc.semaphore("dout"))
        dzout = ctx.enter_context(nc.semaphore("dzout"))

        block = ctx.enter_context(nc.Block())

        # shift matrices in sh: slot d (0..4) = S_d  (S_d[p, o] = 1 iff p == o+d),
        # slot 4+d (d=1..4) = W_d (W_d[p, o] = 1 iff p == o+d-128)
        def S(d):
            return sh[:, 128 * d:128 * (d + 1)]

        def Wr(d):
            return sh[:, 128 * (4 + d):128 * (5 + d)]

        def wcol(f, k):
            return slice(768 * f + 128 * k, 768 * f + 128 * (k + 1))

        @block.sync
        def _(sync):
            sync.dma_start(out=bias_s[:, :], in_=bias_in[:, :]).then_inc(dxh, 16)
            for k in range(0, NK, 2):
                for ph in range(2):
                    rs = slice(128 * k + 64 * ph, 128 * k + 64 * (ph + 1))
                    sync.dma_start(out=xh[k][64 * ph:64 * (ph + 1), :],
                                   in_=xh_in[rs, :]).then_inc(dxh, 16)
            # weight quarters (f pairs), chained so per-quarter gates are
            # ordered. Q0's wh is issued by the scalar queue on its own
            # semaphore so the first main matmul term can start early.
            for f in (0, 1):
                for hb in range(2):
                    cs = slice(768 * f + 384 * hb, 768 * f + 384 * (hb + 1))
                    sync.dma_start(out=wl_s[:, cs], in_=wl_in[:, cs]).then_inc(dwq, 16)
            for q in range(1, 4):
                sync.wait_ge(dw0, 16 * 4)
                sync.wait_ge(dwq, 16 * (4 + 8 * (q - 1)))
                for f in (2 * q, 2 * q + 1):
                    for hb in range(2):
                        cs = slice(768 * f + 384 * hb, 768 * f + 384 * (hb + 1))
                        sync.dma_start(out=wh_s[:, cs], in_=wh_in[:, cs]).then_inc(dwq, 16)
                        sync.dma_start(out=wl_s[:, cs], in_=wl_in[:, cs]).then_inc(dwq, 16)
            ndc = 0
            for i in range(NF):
                # split the last two tiles' transfers so the final drain is short
                parts = 2 if i >= 6 else 1
                sync.wait_ge(sCcpS, i + 1)
                for p in range(parts):
                    w = 256 // parts
                    sync.dma_start(out=ct_out[128 * i:128 * (i + 1), w * p:w * (p + 1)],
                                   in_=cc[i][:, w * p:w * (p + 1)]).then_inc(dout, 16)
                    ndc += 1
                sync.wait_ge(sCcpV, i + 1)
                for p in range(parts):
                    w = 256 // parts
                    sync.dma_start(out=ct_out[128 * i:128 * (i + 1), 256 + w * p:256 + w * (p + 1)],
                                   in_=cc[i][:, 256 + w * p:256 + w * (p + 1)]).then_inc(dout, 16)
                    ndc += 1
            sync.wait_ge(dout, 16 * (8 + ndc))
            sync.wait_ge(dzout, 16 * 8)

        @block.tensor
        def _(tensor):
            def stage_a(idx):
                # A_idx[o, v] = sum_{d=0..3} z[(128*idx + o + d) % 1024, v - d], v in [3,519)
                if idx == 0:
                    tensor.wait_ge(dsh, 16 * 3)           # shift matrices
                if idx >= 2:
                    tensor.wait_ge(sAcp, 2 * (idx - 1))   # slot free of A_{idx-2}
                if idx >= 3:
                    tensor.wait_ge(sCcpS, idx - 2)        # slot free of B_{idx-3}
                    tensor.wait_ge(sCcpV, idx - 2)
                tensor.wait_ge(szz, min(idx + 2, NF))
                p0 = pA[2 * (idx % 2)]
                p1 = pA[2 * (idx % 2) + 1]
                for d in range(4):
                    for half, p in ((0, p0), (1, p1)):
                        c0 = 3 + 258 * half - d
                        tensor.matmul(p[:, :], lhsT=S(d), rhs=z_s[idx][:, c0:c0 + 258],
                                      start=(d == 0), stop=False)
                for d in range(1, 4):
                    for half, p in ((0, p0), (1, p1)):
                        c0 = 3 + 258 * half - d
                        ins = tensor.matmul(p[:, :], lhsT=Wr(d),
                                            rhs=z_s[(idx + 1) % NF][:, c0:c0 + 258],
                                            start=False, stop=(d == 3))
                        if d == 3:
                            ins.then_inc(sAmm, 1)

            def stage_b(i):
                # C_i[o, u] = A[128i+o, u+7] + A[(128i+o+4)%1024, u+3], u in [0,512)
                # A_s[i][:, j] holds A[128i + ., j+3]. Borrows pA slot (i+1)%2.
                tensor.wait_ge(sAcp, 2 * min(i + 2, NF))
                if i == 7:
                    tensor.wait_ge(sCcpS, 6)   # slot 0 free of B_5
                    tensor.wait_ge(sCcpV, 6)
                s = (i + 1) % 2
                bA, bB = pA[2 * s], pA[2 * s + 1]
                for mat, rhs_off in ((S(0), 4), (S(4), 0)):
                    for bank, c0 in ((bA, 0), (bB, 256)):
                        tensor.matmul(bank[:, 0:256], lhsT=mat,
                                      rhs=A_s[i][:, c0 + rhs_off:c0 + rhs_off + 256],
                                      start=(rhs_off == 4), stop=False)
                for bank, c0 in ((bA, 0), (bB, 256)):
                    ins = tensor.matmul(bank[:, 0:256], lhsT=Wr(4),
                                        rhs=A_s[(i + 1) % NF][:, c0:c0 + 256],
                                        start=False, stop=True)
                    ins.then_inc(sBmm, 1)

            for f in range(NF):
                if f == 0:
                    tensor.wait_ge(dxh, 16 * 7)
                    tensor.wait_ge(dxg, 16 * 6)
                    tensor.wait_ge(dw0, 16 * 4)
                if f >= 2:
                    tensor.wait_ge(dwq, 16 * (4 + 8 * (f // 2)))
                    tensor.wait_ge(sza, 2 * (f - 1))  # pingpong banks free
                bA, bB = mA[f % 2], mB[f % 2]
                csA, csB = slice(0, 260), slice(260, 520)
                for k in range(NK):
                    tensor.matmul(bA[:, :], lhsT=wh_s[:, wcol(f, k)], rhs=xh[k][:, csA],
                                  start=(k == 0), stop=False)
                    tensor.matmul(bB[:, :], lhsT=wh_s[:, wcol(f, k)], rhs=xh[k][:, csB],
                                  start=(k == 0), stop=False)
                if f == 0:
                    tensor.wait_ge(dxl, 16 * 6)
                for k in range(NK):
                    tensor.matmul(bA[:, :], lhsT=wh_s[:, wcol(f, k)], rhs=xl[k][:, csA],
                                  start=False, stop=False)
                    tensor.matmul(bB[:, :], lhsT=wh_s[:, wcol(f, k)], rhs=xl[k][:, csB],
                                  start=False, stop=False)
                if f <= 1:
                    tensor.wait_ge(dwq, 16 * 4)       # Q0's wl half
                for k in range(NK):
                    last = k == NK - 1
                    ins = tensor.matmul(bA[:, :], lhsT=wl_s[:, wcol(f, k)], rhs=xh[k][:, csA],
                                        start=False, stop=last)
                    if last:
                        ins.then_inc(smm, 1)
                    ins = tensor.matmul(bB[:, :], lhsT=wl_s[:, wcol(f, k)], rhs=xh[k][:, csB],
                                        start=False, stop=last)
                    if last:
                        ins.then_inc(smm, 1)
                # conv stages trail the main matmuls by 2 / 4 tiles
                if f >= 2:
                    stage_a(f - 2)
                if f >= 4:
                    stage_b(f - 4)
            stage_b(4)
            stage_a(6)
            stage_a(7)
            stage_b(5)
            stage_b(6)
            stage_b(7)

        @block.scalar
        def _(scalar):
            for f in (0, 1):
                for hb in range(2):
                    cs = slice(768 * f + 384 * hb, 768 * f + 384 * (hb + 1))
                    scalar.dma_start(out=wh_s[:, cs], in_=wh_in[:, cs]).then_inc(dw0, 16)
            for k in range(NK):
                scalar.dma_start(out=xl[k][:, :], in_=xl_in[128 * k:128 * (k + 1), :]).then_inc(dxl, 16)

            def c_copy_s(i):
                s = (i + 1) % 2
                scalar.wait_ge(sBmm, 2 * i + 1)
                scalar.activation(out=cc[i][:, 0:256], in_=pA[2 * s][:, 0:256],
                                  func=mybir.ActivationFunctionType.Copy).then_inc(sCcpS, 1)

            for f in range(NF):
                bias_l = bias_s[:, f:f + 1]
                for half, bank in ((0, mA[f % 2]), (1, mB[f % 2])):
                    scalar.wait_ge(smm, 2 * f + half + 1)
                    scalar.activation(out=a_s[f][:, 260 * half:260 * (half + 1)], in_=bank[:, :],
                                      bias=bias_l,
                                      func=mybir.ActivationFunctionType.Tanh).then_inc(sza, 1)
                scalar.wait_ge(sza, 2 * (f + 1))
                scalar.dma_start(out=at_out[128 * f:128 * (f + 1), :],
                                 in_=a_s[f][:, 3:515]).then_inc(dout, 16)
                if f >= 4:
                    c_copy_s(f - 4)
            for i in range(4, NF):
                c_copy_s(i)

        @block.gpsimd
        def _(gpsimd):
            for k in range(1, NK, 2):
                for ph in range(2):
                    rs = slice(128 * k + 64 * ph, 128 * k + 64 * (ph + 1))
                    gpsimd.dma_start(out=xh[k][64 * ph:64 * (ph + 1), :],
                                     in_=xh_in[rs, :]).then_inc(dxg, 16)
            for j in range(3):
                gpsimd.dma_start(out=sh[:, 384 * j:384 * (j + 1)],
                                 in_=sh_in[:, 384 * j:384 * (j + 1)]).then_inc(dsh, 16)
            for f in range(NF):
                gpsimd.wait_ge(szz, f + 1)
                gpsimd.dma_start(out=zt_out[128 * f:128 * (f + 1), :],
                                 in_=z_s[f][:, 3:515]).then_inc(dzout, 16)

        @block.vector
        def _(vector):
            def copy_a(idx):
                for half in range(2):
                    vector.wait_ge(sAmm, 2 * idx + half + 1)
                    vector.tensor_scalar_add(out=A_s[idx][:, 258 * half:258 * (half + 1)],
                                             in0=pA[2 * (idx % 2) + half][:, :],
                                             scalar1=0.0).then_inc(sAcp, 1)

            def c_copy_v(i):
                s = (i + 1) % 2
                vector.wait_ge(sBmm, 2 * i + 2)
                vector.tensor_scalar_add(out=cc[i][:, 256:512], in0=pA[2 * s + 1][:, 0:256],
                                         scalar1=0.0).then_inc(sCcpV, 1)

            for f in range(NF):
                vector.wait_ge(sza, 2 * (f + 1))
                vector.tensor_scalar(out=z_s[f][:, :], in0=a_s[f][:, :], scalar1=0.0,
                                     scalar2=None, op0=mybir.AluOpType.is_gt).then_inc(szz, 1)
                if f >= 2:
                    copy_a(f - 2)
                if f >= 4:
                    c_copy_v(f - 4)
            copy_a(6)
            c_copy_v(4)
            copy_a(7)
            c_copy_v(5)
            c_copy_v(6)
            c_copy_v(7)

    return nc


_NC = None


def _prep_weights(W, b):
    wh4 = W.astype(bf16_np)
    wl4 = (W - wh4.astype(np.float32)).astype(bf16_np)

    # pack [768, 1024] -> [128, (f, k, o)]: col = 768f + 128k + o
    def pack(w):
        return np.ascontiguousarray(
            w.reshape(NK, 128, NF, 128).transpose(1, 2, 0, 3).reshape(128, NF * NK * 128))
    biasv = np.ascontiguousarray(b.reshape(NF, 128).T.astype(np.float32))  # [128, 8]

    # S_d[p, o] = 1 iff p == o + d; W_d[p, o] = 1 iff p == o + d - 128
    s = np.zeros((128, 9 * 128), dtype=bf16_np)
    for d in range(5):
        for o in range(0, 128 - d):
            s[o + d, 128 * d + o] = 1
    for d in range(1, 5):
        for o in range(128 - d, 128):
            s[o + d - 128, 128 * (4 + d) + o] = 1
    return pack(wh4), pack(wl4), biasv, s


def kernel(x: np.ndarray, W: np.ndarray, b: np.ndarray):
    global _NC, LAST_RESULTS
    x = np.asarray(x, dtype=np.float32)
    W = np.asarray(W, dtype=np.float32)
    b = np.asarray(b, dtype=np.float32)

    if _NC is None:
        _NC = _build_module()
    nc = _NC

    wh_p, wl_p, biasv, shifts = _prep_weights(W, b)

    in_maps = []
    metas = []
    for c in range(8):
        bi, h = c // 2, c % 2
        t0 = OWN * h
        lo, hi = t0 - 3, t0 + 517  # 520 rows
        xc = np.zeros((T, IN_DIM), dtype=np.float32)
        src_lo, src_hi = max(lo, 0), min(hi, SEQ)
        xc[src_lo - lo:src_hi - lo, :] = x[bi, src_lo:src_hi, :]
        xt = np.ascontiguousarray(xc.T)              # [768, 520] f32
        xhc = xt.astype(bf16_np)
        xlc = (xt - xhc.astype(np.float32)).astype(bf16_np)
        in_maps.append({"xh": xhc, "xl": xlc, "wh": wh_p, "wl": wl_p,
                        "biasv": biasv, "sh": shifts})
        metas.append((bi, t0))

    res = run_bass_kernel_spmd(nc, in_maps, list(range(8)))
    LAST_RESULTS = res

    a_full = np.empty((BATCH, SEQ, F_DIM), dtype=np.float32)
    z_full = np.empty((BATCH, SEQ, F_DIM), dtype=np.float32)
    zc_full = np.empty((BATCH, SEQ, F_DIM), dtype=np.float32)
    for c in range(8):
        bi, t0 = metas[c]
        r = res.results[c]
        a_full[bi, t0:t0 + OWN, :] = r["at"].astype(np.float32).T
        z_full[bi, t0:t0 + OWN, :] = r["zt"].astype(np.float32).T
        zc_full[bi, t0:t0 + OWN, :] = r["ct"].astype(np.float32).T

    # Boundary fixup: device z at t<0 / t>=SEQ is (tanh(b) > 0) instead of the
    # conv's zero padding, affecting z_conv at t in [0,3) and [SEQ-4, SEQ).
    # Recompute those columns from the exact device z (no-op when b == 0).
    for t in list(range(3)) + list(range(SEQ - 4, SEQ)):
        acc = np.zeros((BATCH, F_DIM), dtype=np.float32)
        for d in range(8):
            ts = t + 4 - d
            if 0 <= ts < SEQ:
                acc += np.roll(z_full[:, ts, :], -d, axis=-1)
        zc_full[:, t, :] = acc
    return (a_full, z_full, zc_full)


# revision 41
# speedup vs baseline: 1.0192x; 1.0192x over previous
"""ConvShiftLayer TRN2 kernel: a = tanh(x@W+b); z = (a>0); z_conv = shift-conv(z).

Math: z_conv[t, o] = sum_{d=0..7} z[t+4-d, (o+d) % 1024]  (zero outside seq range)
Factored: A[p, v] = sum_{d0=0..3} z[v-d0, (p+d0)%F]; C[o, u] = A[o, u+7] + A[(o+4)%F, u+3].

Sharding: 8 cores = (batch 4) x (seq halves 2); each core computes a 512-step seq
slice + halo (3 left / 4 right) from scratch. On-chip layout: features on
partitions (8 tiles of 128), seq on the free axis (520 cols: 3 halo + 512 owned
+ 4 halo + 1 pad).

Main matmul: 3-term bf16 split (xh@wh + xl@wh + xh@wl) accumulated in fp32 psum
(error ~2^-16, exact enough for the z threshold). Bias enters through the
activation instruction's per-partition bias operand; halo columns outside the
sequence get pre-activation 0 -> z = (tanh(b) > 0), which is wrong only for
t<0 / t>=1024 when b != 0, and the host fixes those 7 boundary z_conv columns
from the (exact) device z.

Conv: banded shift-matrix matmuls in bf16 (exact on 0/1 data) accumulated in
psum. Stage A applies (I + Q + Q^2 + Q^3), stage B (I + Q^4); Q's feature shift
is the lhsT matrix (in-tile band + cross-tile wrap into tile f+1), its time
shift a free-axis rhs column offset. Stage B runs interleaved with the main
matmuls, borrowing the stage-A psum slot its gate proves free. No DRAM round
trips.

DMA strategy (each DMA instruction costs ~620ns to issue and one queue moves
~22GB/s): input issue is spread over four engine queues (sync: bias + xh-even +
weight quarters, chained so per-quarter gates are race-free; gpsimd: shift
matrices + xh-odd; scalar: xl; vector: first wl quarter). Producer engines
issue their own outputs (scalar: a; gpsimd: z; sync drains c as copies land).
Outputs are bf16 (a ~0.2% rounding; z, z_conv exact); host converts to f32.
"""
import numpy as np
import ml_dtypes
from contextlib import ExitStack

import concourse.bass as bass
import concourse.mybir as mybir
from concourse.bass_utils import run_bass_kernel_spmd

F_DIM = 1024
IN_DIM = 768
SEQ = 1024
BATCH = 4
T = 520          # 3 halo + 512 owned + 4 halo + 1 pad
OWN = 512
NK = 6           # K tiles of 128 over 768
NF = 8           # feature tiles of 128

f32 = mybir.dt.float32
bf16 = mybir.dt.bfloat16
bf16_np = ml_dtypes.bfloat16

LAST_RESULTS = None  # BassKernelResults of the most recent run (for test.py)


def _build_module():
    nc = bass.Bass()
    xh_in = nc.declare_dram_parameter("xh", [NK * 128, T], bf16, isOutput=False)
    xl_in = nc.declare_dram_parameter("xl", [NK * 128, T], bf16, isOutput=False)
    wh_in = nc.declare_dram_parameter("wh", [128, NF * NK * 128], bf16, isOutput=False)
    wl_in = nc.declare_dram_parameter("wl", [128, NF * NK * 128], bf16, isOutput=False)
    bias_in = nc.declare_dram_parameter("biasv", [128, NF], f32, isOutput=False)
    sh_in = nc.declare_dram_parameter("sh", [128, 9 * 128], bf16, isOutput=False)
    at_out = nc.declare_dram_parameter("at", [F_DIM, OWN], bf16, isOutput=True)
    zt_out = nc.declare_dram_parameter("zt", [F_DIM, OWN], bf16, isOutput=True)
    ct_out = nc.declare_dram_parameter("ct", [F_DIM, OWN], bf16, isOutput=True)

    ctx = ExitStack()
    with ctx:
        xh = [ctx.enter_context(nc.sbuf_tensor(f"xhs{k}", [128, T], bf16)) for k in range(NK)]
        xl = [ctx.enter_context(nc.sbuf_tensor(f"xls{k}", [128, T], bf16)) for k in range(NK)]
        wh_s = ctx.enter_context(nc.sbuf_tensor("whs", [128, NF * NK * 128], bf16))
        wl_s = ctx.enter_context(nc.sbuf_tensor("wls", [128, NF * NK * 128], bf16))
        bias_s = ctx.enter_context(nc.sbuf_tensor("biasvs", [128, NF], f32))
        sh = ctx.enter_context(nc.sbuf_tensor("shs", [128, 9 * 128], bf16))
        a_s = [ctx.enter_context(nc.sbuf_tensor(f"a{f}", [128, T], bf16)) for f in range(NF)]
        z_s = [ctx.enter_context(nc.sbuf_tensor(f"z{f}", [128, T], bf16)) for f in range(NF)]
        A_s = [ctx.enter_context(nc.sbuf_tensor(f"A{f}", [128, 516], bf16)) for f in range(NF)]
        cc = [ctx.enter_context(nc.sbuf_tensor(f"cc{f}", [128, OWN], bf16)) for f in range(NF)]
        # psum: 4 banks main matmul (pingpong pairs of 260+260), 4 banks conv
        # (stage A pingpong pairs of 258+258; stage B borrows the slot its
        # sAcp gate proves free).
        mA = [ctx.enter_context(nc.psum_tensor(f"mA{i}", [128, 260], f32)) for i in range(2)]
        mB = [ctx.enter_context(nc.psum_tensor(f"mB{i}", [128, 260], f32)) for i in range(2)]
        pA = [ctx.enter_context(nc.psum_tensor(f"pA{i}", [128, 258], f32)) for i in range(4)]

        dxh = ctx.enter_context(nc.semaphore("dxh"))
        dxg = ctx.enter_context(nc.semaphore("dxg"))
        dsh = ctx.enter_context(nc.semaphore("dsh"))
        dxl = ctx.enter_context(nc.semaphore("dxl"))
        dw0 = ctx.enter_context(nc.semaphore("dw0"))
        dwq = ctx.enter_context(nc.semaphore("dwq"))
        smm = ctx.enter_context(nc.semaphore("smm"))
        sza = ctx.enter_context(nc.semaphore("sza"))
        szz = ctx.enter_context(nc.semaphore("szz"))
        sAmm = ctx.enter_context(nc.semaphore("sAmm"))
        sAcp = ctx.enter_context(nc.semaphore("sAcp"))
        sBmm = ctx.enter_context(nc.semaphore("sBmm"))
        sCcpS = ctx.enter_context(nc.semaphore("sCcpS"))
        sCcpV = ctx.enter_context(nc.semaphore("sCcpV"))
        dout = ctx.enter_context(nc.semaphore("dout"))
        dzout = ctx.enter_context(nc.semaphore("dzout"))

        block = ctx.enter_context(nc.Block())

        # shift matrices in sh: slot d (0..4) = S_d  (S_d[p, o] = 1 iff p == o+d),
        # slot 4+d (d=1..4) = W_d (W_d[p, o] = 1 iff p == o+d-128)
        def S(d):
            return sh[:, 128 * d:128 * (d + 1)]

        def Wr(d):
            return sh[:, 128 * (4 + d):128 * (5 + d)]

        def wcol(f, k):
            return slice(768 * f + 128 * k, 768 * f + 128 * (k + 1))

        @block.sync
        def _(sync):
            sync.dma_start(out=bias_s[:, :], in_=bias_in[:, :]).then_inc(dxh, 16)
            for k in range(0, NK, 2):
                sync.dma_start(out=xh[k][:, :], in_=xh_in[128 * k:128 * (k + 1), :]).then_inc(dxh, 16)
            # weight quarters (f pairs), chained so per-quarter gates are
            # ordered. Q0's wh goes on its own semaphore so the first main
            # matmul term can start before Q0's wl lands.
            for f in (0, 1):
                for hb in range(2):
                    cs = slice(768 * f + 384 * hb, 768 * f + 384 * (hb + 1))
                    sync.dma_start(out=wh_s[:, cs], in_=wh_in[:, cs]).then_inc(dw0, 16)
            for f in (0, 1):
                for hb in range(2):
                    cs = slice(768 * f + 384 * hb, 768 * f + 384 * (hb + 1))
                    sync.dma_start(out=wl_s[:, cs], in_=wl_in[:, cs]).then_inc(dwq, 16)
            for q in range(1, 4):
                sync.wait_ge(dw0, 16 * 4)
                sync.wait_ge(dwq, 16 * (4 + 8 * (q - 1)))
                for f in (2 * q, 2 * q + 1):
                    for hb in range(2):
                        cs = slice(768 * f + 384 * hb, 768 * f + 384 * (hb + 1))
                        sync.dma_start(out=wh_s[:, cs], in_=wh_in[:, cs]).then_inc(dwq, 16)
                        sync.dma_start(out=wl_s[:, cs], in_=wl_in[:, cs]).then_inc(dwq, 16)
            for i in range(NF):
                sync.wait_ge(sCcpS, i + 1)
                sync.dma_start(out=ct_out[128 * i:128 * (i + 1), 0:256],
                               in_=cc[i][:, 0:256]).then_inc(dout, 16)
                sync.wait_ge(sCcpV, i + 1)
                sync.dma_start(out=ct_out[128 * i:128 * (i + 1), 256:512],
                               in_=cc[i][:, 256:512]).then_inc(dout, 16)
            sync.wait_ge(dout, 16 * 24)
            sync.wait_ge(dzout, 16 * 8)

        @block.tensor
        def _(tensor):
            def stage_a(idx):
                # A_idx[o, v] = sum_{d=0..3} z[(128*idx + o + d) % 1024, v - d], v in [3,519)
                if idx == 0:
                    tensor.wait_ge(dsh, 16 * 3)           # shift matrices
                if idx >= 2:
                    tensor.wait_ge(sAcp, 2 * (idx - 1))   # slot free of A_{idx-2}
                if idx >= 3:
                    tensor.wait_ge(sCcpS, idx - 2)        # slot free of B_{idx-3}
                    tensor.wait_ge(sCcpV, idx - 2)
                tensor.wait_ge(szz, min(idx + 2, NF))
                p0 = pA[2 * (idx % 2)]
                p1 = pA[2 * (idx % 2) + 1]
                for d in range(4):
                    for half, p in ((0, p0), (1, p1)):
                        c0 = 3 + 258 * half - d
                        tensor.matmul(p[:, :], lhsT=S(d), rhs=z_s[idx][:, c0:c0 + 258],
                                      start=(d == 0), stop=False)
                for d in range(1, 4):
                    for half, p in ((0, p0), (1, p1)):
                        c0 = 3 + 258 * half - d
                        ins = tensor.matmul(p[:, :], lhsT=Wr(d),
                                            rhs=z_s[(idx + 1) % NF][:, c0:c0 + 258],
                                            start=False, stop=(d == 3))
                        if d == 3:
                            ins.then_inc(sAmm, 1)

            def stage_b(i):
                # C_i[o, u] = A[128i+o, u+7] + A[(128i+o+4)%1024, u+3], u in [0,512)
                # A_s[i][:, j] holds A[128i + ., j+3]. Borrows pA slot (i+1)%2.
                tensor.wait_ge(sAcp, 2 * min(i + 2, NF))
                if i == 7:
                    tensor.wait_ge(sCcpS, 6)   # slot 0 free of B_5
                    tensor.wait_ge(sCcpV, 6)
                s = (i + 1) % 2
                bA, bB = pA[2 * s], pA[2 * s + 1]
                for mat, rhs_off in ((S(0), 4), (S(4), 0)):
                    for bank, c0 in ((bA, 0), (bB, 256)):
                        tensor.matmul(bank[:, 0:256], lhsT=mat,
                                      rhs=A_s[i][:, c0 + rhs_off:c0 + rhs_off + 256],
                                      start=(rhs_off == 4), stop=False)
                for bank, c0 in ((bA, 0), (bB, 256)):
                    ins = tensor.matmul(bank[:, 0:256], lhsT=Wr(4),
                                        rhs=A_s[(i + 1) % NF][:, c0:c0 + 256],
                                        start=False, stop=True)
                    ins.then_inc(sBmm, 1)

            for f in range(NF):
                if f == 0:
                    tensor.wait_ge(dxh, 16 * 4)
                    tensor.wait_ge(dxg, 16 * 3)
                    tensor.wait_ge(dw0, 16 * 4)
                if f >= 2:
                    tensor.wait_ge(dwq, 16 * (4 + 8 * (f // 2)))
                    tensor.wait_ge(sza, 2 * (f - 1))  # pingpong banks free
                bA, bB = mA[f % 2], mB[f % 2]
                csA, csB = slice(0, 260), slice(260, 520)
                for k in range(NK):
                    tensor.matmul(bA[:, :], lhsT=wh_s[:, wcol(f, k)], rhs=xh[k][:, csA],
                                  start=(k == 0), stop=False)
                    tensor.matmul(bB[:, :], lhsT=wh_s[:, wcol(f, k)], rhs=xh[k][:, csB],
                                  start=(k == 0), stop=False)
                if f == 0:
                    tensor.wait_ge(dxl, 16 * 6)
                for k in range(NK):
                    tensor.matmul(bA[:, :], lhsT=wh_s[:, wcol(f, k)], rhs=xl[k][:, csA],
                                  start=False, stop=False)
                    tensor.matmul(bB[:, :], lhsT=wh_s[:, wcol(f, k)], rhs=xl[k][:, csB],
                                  start=False, stop=False)
                if f <= 1:
                    tensor.wait_ge(dwq, 16 * 4)       # Q0's wl half
                for k in range(NK):
                    last = k == NK - 1
                    ins = tensor.matmul(bA[:, :], lhsT=wl_s[:, wcol(f, k)], rhs=xh[k][:, csA],
                                        start=False, stop=last)
                    if last:
                        ins.then_inc(smm, 1)
                    ins = tensor.matmul(bB[:, :], lhsT=wl_s[:, wcol(f, k)], rhs=xh[k][:, csB],
                                        start=False, stop=last)
                    if last:
                        ins.then_inc(smm, 1)
                # conv stages trail the main matmuls by 2 / 4 tiles
                if f >= 2:
                    stage_a(f - 2)
                if f >= 4:
                    stage_b(f - 4)
            stage_a(6)
            stage_b(4)
            stage_a(7)
            stage_b(5)
            stage_b(6)
            stage_b(7)

        @block.scalar
        def _(scalar):
            for k in range(NK):
                scalar.dma_start(out=xl[k][:, :], in_=xl_in[128 * k:128 * (k + 1), :]).then_inc(dxl, 16)

            def c_copy_s(i):
                s = (i + 1) % 2
                scalar.wait_ge(sBmm, 2 * i + 1)
                scalar.activation(out=cc[i][:, 0:256], in_=pA[2 * s][:, 0:256],
                                  func=mybir.ActivationFunctionType.Copy).then_inc(sCcpS, 1)

            for f in range(NF):
                bias_l = bias_s[:, f:f + 1]
                for half, bank in ((0, mA[f % 2]), (1, mB[f % 2])):
                    scalar.wait_ge(smm, 2 * f + half + 1)
                    scalar.activation(out=a_s[f][:, 260 * half:260 * (half + 1)], in_=bank[:, :],
                                      bias=bias_l,
                                      func=mybir.ActivationFunctionType.Tanh).then_inc(sza, 1)
                scalar.wait_ge(sza, 2 * (f + 1))
                scalar.dma_start(out=at_out[128 * f:128 * (f + 1), :],
                                 in_=a_s[f][:, 3:515]).then_inc(dout, 16)
                if f >= 4:
                    c_copy_s(f - 4)
            for i in range(4, NF):
                c_copy_s(i)

        @block.gpsimd
        def _(gpsimd):
            for k in range(1, NK, 2):
                gpsimd.dma_start(out=xh[k][:, :], in_=xh_in[128 * k:128 * (k + 1), :]).then_inc(dxg, 16)
            for j in range(3):
                gpsimd.dma_start(out=sh[:, 384 * j:384 * (j + 1)],
                                 in_=sh_in[:, 384 * j:384 * (j + 1)]).then_inc(dsh, 16)
            for f in range(NF):
                gpsimd.wait_ge(szz, f + 1)
                gpsimd.dma_start(out=zt_out[128 * f:128 * (f + 1), :],
                                 in_=z_s[f][:, 3:515]).then_inc(dzout, 16)

        @block.vector
        def _(vector):
            def copy_a(idx):
                for half in range(2):
                    vector.wait_ge(sAmm, 2 * idx + half + 1)
                    vector.tensor_scalar_add(out=A_s[idx][:, 258 * half:258 * (half + 1)],
                                             in0=pA[2 * (idx % 2) + half][:, :],
                                             scalar1=0.0).then_inc(sAcp, 1)

            def c_copy_v(i):
                s = (i + 1) % 2
                vector.wait_ge(sBmm, 2 * i + 2)
                vector.tensor_scalar_add(out=cc[i][:, 256:512], in0=pA[2 * s + 1][:, 0:256],
                                         scalar1=0.0).then_inc(sCcpV, 1)

            for f in range(NF):
                vector.wait_ge(sza, 2 * (f + 1))
                vector.tensor_scalar(out=z_s[f][:, :], in0=a_s[f][:, :], scalar1=0.0,
                                     scalar2=None, op0=mybir.AluOpType.is_gt).then_inc(szz, 1)
                if f >= 2:
                    copy_a(f - 2)
                if f >= 4:
                    c_copy_v(f - 4)
            copy_a(6)
            c_copy_v(4)
            copy_a(7)
            c_copy_v(5)
            c_copy_v(6)
            c_copy_v(7)

    return nc


_NC = None


def _prep_weights(W, b):
    wh4 = W.astype(bf16_np)
    wl4 = (W - wh4.astype(np.float32)).astype(bf16_np)

    # pack [768, 1024] -> [128, (f, k, o)]: col = 768f + 128k + o
    def pack(w):
        return np.ascontiguousarray(
            w.reshape(NK, 128, NF, 128).transpose(1, 2, 0, 3).reshape(128, NF * NK * 128))
    biasv = np.ascontiguousarray(b.reshape(NF, 128).T.astype(np.float32))  # [128, 8]

    # S_d[p, o] = 1 iff p == o + d; W_d[p, o] = 1 iff p == o + d - 128
    s = np.zeros((128, 9 * 128), dtype=bf16_np)
    for d in range(5):
        for o in range(0, 128 - d):
            s[o + d, 128 * d + o] = 1
    for d in range(1, 5):
        for o in range(128 - d, 128):
            s[o + d - 128, 128 * (4 + d) + o] = 1
    return pack(wh4), pack(wl4), biasv, s


def kernel(x: np.ndarray, W: np.ndarray, b: np.ndarray):
    global _NC, LAST_RESULTS
    x = np.asarray(x, dtype=np.float32)
    W = np.asarray(W, dtype=np.float32)
    b = np.asarray(b, dtype=np.float32)

    if _NC is None:
        _NC = _build_module()
    nc = _NC

    wh_p, wl_p, biasv, shifts = _prep_weights(W, b)

    in_maps = []
    metas = []
    for c in range(8):
        bi, h = c // 2, c % 2
        t0 = OWN * h
        lo, hi = t0 - 3, t0 + 517  # 520 rows
        xc = np.zeros((T, IN_DIM), dtype=np.float32)
        src_lo, src_hi = max(lo, 0), min(hi, SEQ)
        xc[src_lo - lo:src_hi - lo, :] = x[bi, src_lo:src_hi, :]
        xt = np.ascontiguousarray(xc.T)              # [768, 520] f32
        xhc = xt.astype(bf16_np)
        xlc = (xt - xhc.astype(np.float32)).astype(bf16_np)
        in_maps.append({"xh": xhc, "xl": xlc, "wh": wh_p, "wl": wl_p,
                        "biasv": biasv, "sh": shifts})
        metas.append((bi, t0))

    res = run_bass_kernel_spmd(nc, in_maps, list(range(8)))
    LAST_RESULTS = res

    a_full = np.empty((BATCH, SEQ, F_DIM), dtype=np.float32)
    z_full = np.empty((BATCH, SEQ, F_DIM), dtype=np.float32)
    zc_full = np.empty((BATCH, SEQ, F_DIM), dtype=np.float32)
    for c in range(8):
        bi, t0 = metas[c]
        r = res.results[c]
        a_full[bi, t0:t0 + OWN, :] = r["at"].astype(np.float32).T
        z_full[bi, t0:t0 + OWN, :] = r["zt"].astype(np.float32).T
        zc_full[bi, t0:t0 + OWN, :] = r["ct"].astype(np.float32).T

    # Boundary fixup: device z at t<0 / t>=SEQ is (tanh(b) > 0) instead of the
    # conv's zero padding, affecting z_conv at t in [0,3) and [SEQ-4, SEQ).
    # Recompute those columns from the exact device z (no-op when b == 0).
    for t in list(range(3)) + list(range(SEQ - 4, SEQ)):
        acc = np.zeros((BATCH, F_DIM), dtype=np.float32)
        for d in range(8):
            ts = t + 4 - d
            if 0 <= ts < SEQ:
                acc += np.roll(z_full[:, ts, :], -d, axis=-1)
        zc_full[:, t, :] = acc
    return (a_full, z_full, zc_full)


# revision 42
# speedup vs baseline: 1.1418x; 1.1203x over previous
"""ConvShiftLayer TRN2 kernel: a = tanh(x@W+b); z = (a>0); z_conv = shift-conv(z).

Math: z_conv[t, o] = sum_{d=0..7} z[t+4-d, (o+d) % 1024]  (zero outside seq range)
Factored: A[p, v] = sum_{d0=0..3} z[v-d0, (p+d0)%F]; C[o, u] = A[o, u+7] + A[(o+4)%F, u+3].

Sharding: 8 cores = (batch 4) x (seq halves 2); each core computes a 512-step seq
slice + halo (3 left / 4 right) from scratch. On-chip layout: features on
partitions (8 tiles of 128), seq on the free axis (520 cols: 3 halo + 512 owned
+ 4 halo + 1 pad).

Main matmul: 3-term bf16 split (xh@wh + xl@wh + xh@wl) accumulated in fp32 psum
(error ~2^-16, exact enough for the z threshold). Bias enters through the
activation instruction's per-partition bias operand; halo columns outside the
sequence get pre-activation 0 -> z = (tanh(b) > 0), which is wrong only for
t<0 / t>=1024 when b != 0, and the host fixes those 7 boundary z_conv columns
from the (exact) device z.

Conv: banded shift-matrix matmuls in bf16 (exact on 0/1 data) accumulated in
psum. Stage A applies (I + Q + Q^2 + Q^3), stage B (I + Q^4); Q's feature shift
is the lhsT matrix (in-tile band + cross-tile wrap into tile f+1), its time
shift a free-axis rhs column offset. Stage B runs interleaved with the main
matmuls, borrowing the stage-A psum slot its gate proves free. No DRAM round
trips.

DMA strategy (each DMA instruction costs ~620ns to issue and one queue moves
~22GB/s): input issue is spread over four engine queues (sync: bias + xh-even +
weight quarters, chained so per-quarter gates are race-free; gpsimd: shift
matrices + xh-odd; scalar: xl; vector: first wl quarter). Producer engines
issue their own outputs (scalar: a; gpsimd: z; sync drains c as copies land).
Outputs are bf16 (a ~0.2% rounding; z, z_conv exact); host converts to f32.
"""
import numpy as np
import ml_dtypes
from contextlib import ExitStack

import concourse.bass as bass
import concourse.mybir as mybir
from concourse.bass_utils import run_bass_kernel_spmd

F_DIM = 1024
IN_DIM = 768
SEQ = 1024
BATCH = 4
T = 520          # 3 halo + 512 owned + 4 halo + 1 pad
OWN = 512
NK = 6           # K tiles of 128 over 768
NF = 8           # feature tiles of 128

f32 = mybir.dt.float32
bf16 = mybir.dt.bfloat16
bf16_np = ml_dtypes.bfloat16

LAST_RESULTS = None  # BassKernelResults of the most recent run (for test.py)


def _build_module():
    nc = bass.Bass()
    xh_in = nc.declare_dram_parameter("xh", [NK * 128, T], bf16, isOutput=False)
    xl_in = nc.declare_dram_parameter("xl", [NK * 128, T], bf16, isOutput=False)
    wh_in = nc.declare_dram_parameter("wh", [128, NF * NK * 128], bf16, isOutput=False)
    wl_in = nc.declare_dram_parameter("wl", [128, NF * NK * 128], bf16, isOutput=False)
    bias_in = nc.declare_dram_parameter("biasv", [128, NF], f32, isOutput=False)
    sh_in = nc.declare_dram_parameter("sh", [128, 9 * 128], bf16, isOutput=False)
    at_out = nc.declare_dram_parameter("at", [F_DIM, OWN], bf16, isOutput=True)
    zt_out = nc.declare_dram_parameter("zt", [F_DIM, OWN], bf16, isOutput=True)
    ct_out = nc.declare_dram_parameter("ct", [F_DIM, OWN], bf16, isOutput=True)

    ctx = ExitStack()
    with ctx:
        xh = [ctx.enter_context(nc.sbuf_tensor(f"xhs{k}", [128, T], bf16)) for k in range(NK)]
        xl = [ctx.enter_context(nc.sbuf_tensor(f"xls{k}", [128, T], bf16)) for k in range(NK)]
        wh_s = ctx.enter_context(nc.sbuf_tensor("whs", [128, NF * NK * 128], bf16))
        wl_s = ctx.enter_context(nc.sbuf_tensor("wls", [128, NF * NK * 128], bf16))
        bias_s = ctx.enter_context(nc.sbuf_tensor("biasvs", [128, NF], f32))
        sh = ctx.enter_context(nc.sbuf_tensor("shs", [128, 9 * 128], bf16))
        a_s = [ctx.enter_context(nc.sbuf_tensor(f"a{f}", [128, T], bf16)) for f in range(NF)]
        z_s = [ctx.enter_context(nc.sbuf_tensor(f"z{f}", [128, T], bf16)) for f in range(NF)]
        A_s = [ctx.enter_context(nc.sbuf_tensor(f"A{f}", [128, 516], bf16)) for f in range(NF)]
        cc = [ctx.enter_context(nc.sbuf_tensor(f"cc{f}", [128, OWN], bf16)) for f in range(NF)]
        # psum: 4 banks main matmul (pingpong pairs of 260+260), 4 banks conv
        # (stage A pingpong pairs of 258+258; stage B borrows the slot its
        # sAcp gate proves free).
        mA = [ctx.enter_context(nc.psum_tensor(f"mA{i}", [128, 260], f32)) for i in range(2)]
        mB = [ctx.enter_context(nc.psum_tensor(f"mB{i}", [128, 260], f32)) for i in range(2)]
        pA = [ctx.enter_context(nc.psum_tensor(f"pA{i}", [128, 258], f32)) for i in range(4)]

        dxh = ctx.enter_context(nc.semaphore("dxh"))
        dxg = ctx.enter_context(nc.semaphore("dxg"))
        dsh = ctx.enter_context(nc.semaphore("dsh"))
        dxl = ctx.enter_context(nc.semaphore("dxl"))
        dw0 = ctx.enter_context(nc.semaphore("dw0"))
        dwq = ctx.enter_context(nc.semaphore("dwq"))
        smm = ctx.enter_context(nc.semaphore("smm"))
        sza = ctx.enter_context(nc.semaphore("sza"))
        szz = ctx.enter_context(nc.semaphore("szz"))
        sAmm = ctx.enter_context(nc.semaphore("sAmm"))
        sAcp = ctx.enter_context(nc.semaphore("sAcp"))
        sBmm = ctx.enter_context(nc.semaphore("sBmm"))
        sCcpS = ctx.enter_context(nc.semaphore("sCcpS"))
        sCcpV = ctx.enter_context(nc.semaphore("sCcpV"))
        dout = ctx.enter_context(nc.semaphore("dout"))
        dzout = ctx.enter_context(nc.semaphore("dzout"))

        block = ctx.enter_context(nc.Block())

        # shift matrices in sh: slot d (0..4) = S_d  (S_d[p, o] = 1 iff p == o+d),
        # slot 4+d (d=1..4) = W_d (W_d[p, o] = 1 iff p == o+d-128)
        def S(d):
            return sh[:, 128 * d:128 * (d + 1)]

        def Wr(d):
            return sh[:, 128 * (4 + d):128 * (5 + d)]

        def wcol(f, k):
            return slice(768 * f + 128 * k, 768 * f + 128 * (k + 1))

        @block.sync
        def _(sync):
            sync.dma_start(out=bias_s[:, :], in_=bias_in[:, :]).then_inc(dxh, 16)
            for k in range(0, NK, 2):
                for ph in range(2):
                    rs = slice(128 * k + 64 * ph, 128 * k + 64 * (ph + 1))
                    sync.dma_start(out=xh[k][64 * ph:64 * (ph + 1), :],
                                   in_=xh_in[rs, :]).then_inc(dxh, 16)
            # weight quarters (f pairs), chained so per-quarter gates are
            # ordered. Q0's wh is issued by the scalar queue on its own
            # semaphore so the first main matmul term can start early.
            for f in (0, 1):
                for hb in range(2):
                    cs = slice(768 * f + 384 * hb, 768 * f + 384 * (hb + 1))
                    sync.dma_start(out=wl_s[:, cs], in_=wl_in[:, cs]).then_inc(dwq, 16)
            for q in range(1, 4):
                sync.wait_ge(dw0, 16 * 4)
                sync.wait_ge(dwq, 16 * (4 + 8 * (q - 1)))
                for f in (2 * q, 2 * q + 1):
                    for hb in range(2):
                        cs = slice(768 * f + 384 * hb, 768 * f + 384 * (hb + 1))
                        sync.dma_start(out=wh_s[:, cs], in_=wh_in[:, cs]).then_inc(dwq, 16)
                        sync.dma_start(out=wl_s[:, cs], in_=wl_in[:, cs]).then_inc(dwq, 16)
            ndc = 0
            for i in range(NF):
                # split the last two tiles' transfers so the final drain is short
                parts = 2 if i >= 6 else 1
                sync.wait_ge(sCcpS, i + 1)
                for p in range(parts):
                    w = 256 // parts
                    sync.dma_start(out=ct_out[128 * i:128 * (i + 1), w * p:w * (p + 1)],
                                   in_=cc[i][:, w * p:w * (p + 1)]).then_inc(dout, 16)
                    ndc += 1
                sync.wait_ge(sCcpV, i + 1)
                for p in range(parts):
                    w = 256 // parts
                    sync.dma_start(out=ct_out[128 * i:128 * (i + 1), 256 + w * p:256 + w * (p + 1)],
                                   in_=cc[i][:, 256 + w * p:256 + w * (p + 1)]).then_inc(dout, 16)
                    ndc += 1
            sync.wait_ge(dout, 16 * (8 + ndc))
            sync.wait_ge(dzout, 16 * 8)

        @block.tensor
        def _(tensor):
            def stage_a(idx):
                # A_idx[o, v] = sum_{d=0..3} z[(128*idx + o + d) % 1024, v - d], v in [3,519)
                if idx == 0:
                    tensor.wait_ge(dsh, 16 * 3)           # shift matrices
                if idx >= 2:
                    tensor.wait_ge(sAcp, 2 * (idx - 1))   # slot free of A_{idx-2}
                if idx >= 3:
                    tensor.wait_ge(sCcpS, idx - 2)        # slot free of B_{idx-3}
                    tensor.wait_ge(sCcpV, idx - 2)
                tensor.wait_ge(szz, min(idx + 2, NF))
                p0 = pA[2 * (idx % 2)]
                p1 = pA[2 * (idx % 2) + 1]
                for d in range(4):
                    for half, p in ((0, p0), (1, p1)):
                        c0 = 3 + 258 * half - d
                        tensor.matmul(p[:, :], lhsT=S(d), rhs=z_s[idx][:, c0:c0 + 258],
                                      start=(d == 0), stop=False)
                for d in range(1, 4):
                    for half, p in ((0, p0), (1, p1)):
                        c0 = 3 + 258 * half - d
                        ins = tensor.matmul(p[:, :], lhsT=Wr(d),
                                            rhs=z_s[(idx + 1) % NF][:, c0:c0 + 258],
                                            start=False, stop=(d == 3))
                        if d == 3:
                            ins.then_inc(sAmm, 1)

            def stage_b(i):
                # C_i[o, u] = A[128i+o, u+7] + A[(128i+o+4)%1024, u+3], u in [0,512)
                # A_s[i][:, j] holds A[128i + ., j+3]. Borrows pA slot (i+1)%2.
                tensor.wait_ge(sAcp, 2 * min(i + 2, NF))
                if i == 7:
                    tensor.wait_ge(sCcpS, 6)   # slot 0 free of B_5
                    tensor.wait_ge(sCcpV, 6)
                s = (i + 1) % 2
                bA, bB = pA[2 * s], pA[2 * s + 1]
                for mat, rhs_off in ((S(0), 4), (S(4), 0)):
                    for bank, c0 in ((bA, 0), (bB, 256)):
                        tensor.matmul(bank[:, 0:256], lhsT=mat,
                                      rhs=A_s[i][:, c0 + rhs_off:c0 + rhs_off + 256],
                                      start=(rhs_off == 4), stop=False)
                for bank, c0 in ((bA, 0), (bB, 256)):
                    ins = tensor.matmul(bank[:, 0:256], lhsT=Wr(4),
                                        rhs=A_s[(i + 1) % NF][:, c0:c0 + 256],
                                        start=False, stop=True)
                    ins.then_inc(sBmm, 1)

            for f in range(NF):
                if f == 0:
                    tensor.wait_ge(dxh, 16 * 7)
                    tensor.wait_ge(dxg, 16 * 6)
                    tensor.wait_ge(dw0, 16 * 4)
                if f >= 2:
                    tensor.wait_ge(dwq, 16 * (4 + 8 * (f // 2)))
                    tensor.wait_ge(sza, 2 * (f - 1))  # pingpong banks free
                bA, bB = mA[f % 2], mB[f % 2]
                csA, csB = slice(0, 260), slice(260, 520)
                for k in range(NK):
                    tensor.matmul(bA[:, :], lhsT=wh_s[:, wcol(f, k)], rhs=xh[k][:, csA],
                                  start=(k == 0), stop=False)
                    tensor.matmul(bB[:, :], lhsT=wh_s[:, wcol(f, k)], rhs=xh[k][:, csB],
                                  start=(k == 0), stop=False)
                if f == 0:
                    tensor.wait_ge(dxl, 16 * 6)
                for k in range(NK):
                    tensor.matmul(bA[:, :], lhsT=wh_s[:, wcol(f, k)], rhs=xl[k][:, csA],
                                  start=False, stop=False)
                    tensor.matmul(bB[:, :], lhsT=wh_s[:, wcol(f, k)], rhs=xl[k][:, csB],
                                  start=False, stop=False)
                if f <= 1:
                    tensor.wait_ge(dwq, 16 * 4)       # Q0's wl half
                for k in range(NK):
                    last = k == NK - 1
                    ins = tensor.matmul(bA[:, :], lhsT=wl_s[:, wcol(f, k)], rhs=xh[k][:, csA],
                                        start=False, stop=last)
                    if last:
                        ins.then_inc(smm, 1)
                    ins = tensor.matmul(bB[:, :], lhsT=wl_s[:, wcol(f, k)], rhs=xh[k][:, csB],
                                        start=False, stop=last)
                    if last:
                        ins.then_inc(smm, 1)
                # conv stages trail the main matmuls by 2 / 4 tiles
                if f >= 2:
                    stage_a(f - 2)
                if f >= 4:
                    stage_b(f - 4)
            stage_b(4)
            stage_a(6)
            stage_a(7)
            stage_b(5)
            stage_b(6)
            stage_b(7)

        @block.scalar
        def _(scalar):
            for f in (0, 1):
                for hb in range(2):
                    cs = slice(768 * f + 384 * hb, 768 * f + 384 * (hb + 1))
                    scalar.dma_start(out=wh_s[:, cs], in_=wh_in[:, cs]).then_inc(dw0, 16)
            for k in range(NK):
                scalar.dma_start(out=xl[k][:, :], in_=xl_in[128 * k:128 * (k + 1), :]).then_inc(dxl, 16)

            def c_copy_s(i):
                s = (i + 1) % 2
                scalar.wait_ge(sBmm, 2 * i + 1)
                scalar.activation(out=cc[i][:, 0:256], in_=pA[2 * s][:, 0:256],
                                  func=mybir.ActivationFunctionType.Copy).then_inc(sCcpS, 1)

            for f in range(NF):
                bias_l = bias_s[:, f:f + 1]
                for half, bank in ((0, mA[f % 2]), (1, mB[f % 2])):
                    scalar.wait_ge(smm, 2 * f + half + 1)
                    scalar.activation(out=a_s[f][:, 260 * half:260 * (half + 1)], in_=bank[:, :],
                                      bias=bias_l,
                                      func=mybir.ActivationFunctionType.Tanh).then_inc(sza, 1)
                scalar.wait_ge(sza, 2 * (f + 1))
                scalar.dma_start(out=at_out[128 * f:128 * (f + 1), :],
                                 in_=a_s[f][:, 3:515]).then_inc(dout, 16)
                if f >= 4:
                    c_copy_s(f - 4)
            for i in range(4, NF):
                c_copy_s(i)

        @block.gpsimd
        def _(gpsimd):
            for k in range(1, NK, 2):
                for ph in range(2):
                    rs = slice(128 * k + 64 * ph, 128 * k + 64 * (ph + 1))
                    gpsimd.dma_start(out=xh[k][64 * ph:64 * (ph + 1), :],
                                     in_=xh_in[rs, :]).then_inc(dxg, 16)
            for j in range(3):
                gpsimd.dma_start(out=sh[:, 384 * j:384 * (j + 1)],
                                 in_=sh_in[:, 384 * j:384 * (j + 1)]).then_inc(dsh, 16)
            for f in range(NF):
                gpsimd.wait_ge(szz, f + 1)
                gpsimd.dma_start(out=zt_out[128 * f:128 * (f + 1), :],
                                 in_=z_s[f][:, 3:515]).then_inc(dzout, 16)

        @block.vector
        def _(vector):
            def copy_a(idx):
                for half in range(2):
                    vector.wait_ge(sAmm, 2 * idx + half + 1)
                    vector.tensor_scalar_add(out=A_s[idx][:, 258 * half:258 * (half + 1)],
                                             in0=pA[2 * (idx % 2) + half][:, :],
                                             scalar1=0.0).then_inc(sAcp, 1)

            def c_copy_v(i):
                s = (i + 1) % 2
                vector.wait_ge(sBmm, 2 * i + 2)
                vector.tensor_scalar_add(out=cc[i][:, 256:512], in0=pA[2 * s + 1][:, 0:256],
                                         scalar1=0.0).then_inc(sCcpV, 1)

            for f in range(NF):
                vector.wait_ge(sza, 2 * (f + 1))
                vector.tensor_scalar(out=z_s[f][:, :], in0=a_s[f][:, :], scalar1=0.0,
                                     scalar2=None, op0=mybir.AluOpType.is_gt).then_inc(szz, 1)
                if f >= 2:
                    copy_a(f - 2)
                if f >= 4:
                    c_copy_v(f - 4)
            copy_a(6)
            c_copy_v(4)
            copy_a(7)
            c_copy_v(5)
            c_copy_v(6)
            c_copy_v(7)

    return nc


_NC = None


def _prep_weights(W, b):
    wh4 = W.astype(bf16_np)
    wl4 = (W - wh4.astype(np.float32)).astype(bf16_np)

    # pack [768, 1024] -> [128, (f, k, o)]: col = 768f + 128k + o
    def pack(w):
        return np.ascontiguousarray(
            w.reshape(NK, 128, NF, 128).transpose(1, 2, 0, 3).reshape(128, NF * NK * 128))
    biasv = np.ascontiguousarray(b.reshape(NF, 128).T.astype(np.float32))  # [128, 8]

    # S_d[p, o] = 1 iff p == o + d; W_d[p, o] = 1 iff p == o + d - 128
    s = np.zeros((128, 9 * 128), dtype=bf16_np)
    for d in range(5):
        for o in range(0, 128 - d):
            s[o + d, 128 * d + o] = 1
    for d in range(1, 5):
        for o in range(128 - d, 128):
            s[o + d - 128, 128 * (4 + d) + o] = 1
    return pack(wh4), pack(wl4), biasv, s


def kernel(x: np.ndarray, W: np.ndarray, b: np.ndarray):
    global _NC, LAST_RESULTS
    x = np.asarray(x, dtype=np.float32)
    W = np.asarray(W, dtype=np.float32)
    b = np.asarray(b, dtype=np.float32)

    if _NC is None:
        _NC = _build_module()
    nc = _NC

    wh_p, wl_p, biasv, shifts = _prep_weights(W, b)

    in_maps = []
    metas = []
    for c in range(8):
        bi, h = c // 2, c % 2
        t0 = OWN * h
        lo, hi = t0 - 3, t0 + 517  # 520 rows
        xc = np.zeros((T, IN_DIM), dtype=np.float32)
        src_lo, src_hi = max(lo, 0), min(hi, SEQ)
        xc[src_lo - lo:src_hi - lo, :] = x[bi, src_lo:src_hi, :]
        xt = np.ascontiguousarray(xc.T)              # [768, 520] f32
        xhc = xt.astype(bf16_np)
        xlc = (xt - xhc.astype(np.float32)).astype(bf16_np)
        in_maps.append({"xh": xhc, "xl": xlc, "wh": wh_p, "wl": wl_p,
                        "biasv": biasv, "sh": shifts})
        metas.append((bi, t0))

    res = run_bass_kernel_spmd(nc, in_maps, list(range(8)))
    LAST_RESULTS = res

    a_full = np.empty((BATCH, SEQ, F_DIM), dtype=np.float32)
    z_full = np.empty((BATCH, SEQ, F_DIM), dtype=np.float32)
    zc_full = np.empty((BATCH, SEQ, F_DIM), dtype=np.float32)
    for c in range(8):
        bi, t0 = metas[c]
        r = res.results[c]
        a_full[bi, t0:t0 + OWN, :] = r["at"].astype(np.float32).T
        z_full[bi, t0:t0 + OWN, :] = r["zt"].astype(np.float32).T
        zc_full[bi, t0:t0 + OWN, :] = r["ct"].astype(np.float32).T

    # Boundary fixup: device z at t<0 / t>=SEQ is (tanh(b) > 0) instead of the
    # conv's zero padding, affecting z_conv at t in [0,3) and [SEQ-4, SEQ).
    # Recompute those columns from the exact device z (no-op when b == 0).
    for t in list(range(3)) + list(range(SEQ - 4, SEQ)):
        acc = np.zeros((BATCH, F_DIM), dtype=np.float32)
        for d in range(8):
            ts = t + 4 - d
            if 0 <= ts < SEQ:
                acc += np.roll(z_full[:, ts, :], -d, axis=-1)
        zc_full[:, t, :] = acc
    return (a_full, z_full, zc_full)


# revision 43
# speedup vs baseline: 1.1580x; 1.0142x over previous
"""ConvShiftLayer TRN2 kernel: a = tanh(x@W+b); z = (a>0); z_conv = shift-conv(z).

Math: z_conv[t, o] = sum_{d=0..7} z[t+4-d, (o+d) % 1024]  (zero outside seq range)
Factored: A[p, v] = sum_{d0=0..3} z[v-d0, (p+d0)%F]; C[o, u] = A[o, u+7] + A[(o+4)%F, u+3].

Sharding: 8 cores = (batch 4) x (seq halves 2); each core computes a 512-step seq
slice + halo (3 left / 4 right) from scratch. On-chip layout: features on
partitions (8 tiles of 128), seq on the free axis (520 cols: 3 halo + 512 owned
+ 4 halo + 1 pad).

Main matmul: 3-term bf16 split (xh@wh + xl@wh + xh@wl) accumulated in fp32 psum
(error ~2^-16, exact enough for the z threshold). Bias enters through the
activation instruction's per-partition bias operand; halo columns outside the
sequence get pre-activation 0 -> z = (tanh(b) > 0), which is wrong only for
t<0 / t>=1024 when b != 0, and the host fixes those 7 boundary z_conv columns
from the (exact) device z.

Conv: banded shift-matrix matmuls in bf16 (exact on 0/1 data) accumulated in
psum. Stage A applies (I + Q + Q^2 + Q^3), stage B (I + Q^4); Q's feature shift
is the lhsT matrix (in-tile band + cross-tile wrap into tile f+1), its time
shift a free-axis rhs column offset. Stage B runs interleaved with the main
matmuls, borrowing the stage-A psum slot its gate proves free. No DRAM round
trips.

DMA strategy (each DMA instruction costs ~620ns to issue and one queue moves
~22GB/s): input issue is spread over four engine queues (sync: bias + xh-even +
weight quarters, chained so per-quarter gates are race-free; gpsimd: shift
matrices + xh-odd; scalar: xl; vector: first wl quarter). Producer engines
issue their own outputs (scalar: a; gpsimd: z; sync drains c as copies land).
Outputs are bf16 (a ~0.2% rounding; z, z_conv exact); host converts to f32.
"""
import numpy as np
import ml_dtypes
from contextlib import ExitStack

import concourse.bass as bass
import concourse.mybir as mybir
from concourse.bass_utils import run_bass_kernel_spmd

F_DIM = 1024
IN_DIM = 768
SEQ = 1024
BATCH = 4
T = 520          # 3 halo + 512 owned + 4 halo + 1 pad
OWN = 512
NK = 6           # K tiles of 128 over 768
NF = 8           # feature tiles of 128

f32 = mybir.dt.float32
bf16 = mybir.dt.bfloat16
bf16_np = ml_dtypes.bfloat16

LAST_RESULTS = None  # BassKernelResults of the most recent run (for test.py)


def _build_module():
    nc = bass.Bass()
    xh_in = nc.declare_dram_parameter("xh", [NK * 128, T], bf16, isOutput=False)
    xl_in = nc.declare_dram_parameter("xl", [NK * 128, T], bf16, isOutput=False)
    wh_in = nc.declare_dram_parameter("wh", [128, NF * NK * 128], bf16, isOutput=False)
    wl_in = nc.declare_dram_parameter("wl", [128, NF * NK * 128], bf16, isOutput=False)
    bias_in = nc.declare_dram_parameter("biasv", [128, NF], f32, isOutput=False)
    sh_in = nc.declare_dram_parameter("sh", [128, 9 * 128], bf16, isOutput=False)
    at_out = nc.declare_dram_parameter("at", [F_DIM, OWN], bf16, isOutput=True)
    zt_out = nc.declare_dram_parameter("zt", [F_DIM, OWN], bf16, isOutput=True)
    ct_out = nc.declare_dram_parameter("ct", [F_DIM, OWN], bf16, isOutput=True)

    ctx = ExitStack()
    with ctx:
        xh = [ctx.enter_context(nc.sbuf_tensor(f"xhs{k}", [128, T], bf16)) for k in range(NK)]
        xl = [ctx.enter_context(nc.sbuf_tensor(f"xls{k}", [128, T], bf16)) for k in range(NK)]
        wh_s = ctx.enter_context(nc.sbuf_tensor("whs", [128, NF * NK * 128], bf16))
        wl_s = ctx.enter_context(nc.sbuf_tensor("wls", [128, NF * NK * 128], bf16))
        bias_s = ctx.enter_context(nc.sbuf_tensor("biasvs", [128, NF], f32))
        sh = ctx.enter_context(nc.sbuf_tensor("shs", [128, 9 * 128], bf16))
        a_s = [ctx.enter_context(nc.sbuf_tensor(f"a{f}", [128, T], bf16)) for f in range(NF)]
        z_s = [ctx.enter_context(nc.sbuf_tensor(f"z{f}", [128, T], bf16)) for f in range(NF)]
        A_s = [ctx.enter_context(nc.sbuf_tensor(f"A{f}", [128, 516], bf16)) for f in range(NF)]
        cc = [ctx.enter_context(nc.sbuf_tensor(f"cc{f}", [128, OWN], bf16)) for f in range(NF)]
        # psum: 4 banks main matmul (pingpong pairs of 260+260), 4 banks conv
        # (stage A pingpong pairs of 258+258; stage B borrows the slot its
        # sAcp gate proves free).
        mA = [ctx.enter_context(nc.psum_tensor(f"mA{i}", [128, 260], f32)) for i in range(2)]
        mB = [ctx.enter_context(nc.psum_tensor(f"mB{i}", [128, 260], f32)) for i in range(2)]
        pA = [ctx.enter_context(nc.psum_tensor(f"pA{i}", [128, 258], f32)) for i in range(4)]

        dxh = ctx.enter_context(nc.semaphore("dxh"))
        dxg = ctx.enter_context(nc.semaphore("dxg"))
        dsh = ctx.enter_context(nc.semaphore("dsh"))
        dxl = ctx.enter_context(nc.semaphore("dxl"))
        dw0 = ctx.enter_context(nc.semaphore("dw0"))
        dwq = ctx.enter_context(nc.semaphore("dwq"))
        smm = ctx.enter_context(nc.semaphore("smm"))
        sza = ctx.enter_context(nc.semaphore("sza"))
        szz = ctx.enter_context(nc.semaphore("szz"))
        sAmm = ctx.enter_context(nc.semaphore("sAmm"))
        sAcp = ctx.enter_context(nc.semaphore("sAcp"))
        sBmm = ctx.enter_context(nc.semaphore("sBmm"))
        sCcpS = ctx.enter_context(nc.semaphore("sCcpS"))
        sCcpV = ctx.enter_context(nc.semaphore("sCcpV"))
        dout = ctx.enter_context(nc.semaphore("dout"))
        dzout = ctx.enter_context(nc.semaphore("dzout"))

        block = ctx.enter_context(nc.Block())

        # shift matrices in sh: slot d (0..4) = S_d  (S_d[p, o] = 1 iff p == o+d),
        # slot 4+d (d=1..4) = W_d (W_d[p, o] = 1 iff p == o+d-128)
        def S(d):
            return sh[:, 128 * d:128 * (d + 1)]

        def Wr(d):
            return sh[:, 128 * (4 + d):128 * (5 + d)]

        def wcol(f, k):
            return slice(768 * f + 128 * k, 768 * f + 128 * (k + 1))

        @block.sync
        def _(sync):
            sync.dma_start(out=bias_s[:, :], in_=bias_in[:, :]).then_inc(dxh, 16)
            for k in range(0, NK, 2):
                for ph in range(2):
                    rs = slice(128 * k + 64 * ph, 128 * k + 64 * (ph + 1))
                    sync.dma_start(out=xh[k][64 * ph:64 * (ph + 1), :],
                                   in_=xh_in[rs, :]).then_inc(dxh, 16)
            # weight quarters (f pairs), chained so per-quarter gates are
            # ordered. Q0's wh is issued by the scalar queue on its own
            # semaphore so the first main matmul term can start early.
            for f in (0, 1):
                for hb in range(2):
                    cs = slice(768 * f + 384 * hb, 768 * f + 384 * (hb + 1))
                    sync.dma_start(out=wl_s[:, cs], in_=wl_in[:, cs]).then_inc(dwq, 16)
            for q in range(1, 4):
                sync.wait_ge(dw0, 16 * 4)
                sync.wait_ge(dwq, 16 * (4 + 8 * (q - 1)))
                for f in (2 * q, 2 * q + 1):
                    for hb in range(2):
                        cs = slice(768 * f + 384 * hb, 768 * f + 384 * (hb + 1))
                        sync.dma_start(out=wh_s[:, cs], in_=wh_in[:, cs]).then_inc(dwq, 16)
                        sync.dma_start(out=wl_s[:, cs], in_=wl_in[:, cs]).then_inc(dwq, 16)
            ndc = 0
            for i in range(NF):
                # split the last two tiles' transfers so the final drain is short
                parts = 2 if i >= 6 else 1
                sync.wait_ge(sCcpS, i + 1)
                for p in range(parts):
                    w = 256 // parts
                    sync.dma_start(out=ct_out[128 * i:128 * (i + 1), w * p:w * (p + 1)],
                                   in_=cc[i][:, w * p:w * (p + 1)]).then_inc(dout, 16)
                    ndc += 1
                sync.wait_ge(sCcpV, i + 1)
                for p in range(parts):
                    w = 256 // parts
                    sync.dma_start(out=ct_out[128 * i:128 * (i + 1), 256 + w * p:256 + w * (p + 1)],
                                   in_=cc[i][:, 256 + w * p:256 + w * (p + 1)]).then_inc(dout, 16)
                    ndc += 1
            sync.wait_ge(dout, 16 * (8 + ndc))
            sync.wait_ge(dzout, 16 * 8)

        @block.tensor
        def _(tensor):
            def stage_a(idx):
                # A_idx[o, v] = sum_{d=0..3} z[(128*idx + o + d) % 1024, v - d], v in [3,519)
                if idx == 0:
                    tensor.wait_ge(dsh, 16 * 3)           # shift matrices
                if idx >= 2:
                    tensor.wait_ge(sAcp, 2 * (idx - 1))   # slot free of A_{idx-2}
                if idx >= 3:
                    tensor.wait_ge(sCcpS, idx - 2)        # slot free of B_{idx-3}
                    tensor.wait_ge(sCcpV, idx - 2)
                tensor.wait_ge(szz, min(idx + 2, NF))
                p0 = pA[2 * (idx % 2)]
                p1 = pA[2 * (idx % 2) + 1]
                for d in range(4):
                    for half, p in ((0, p0), (1, p1)):
                        c0 = 3 + 258 * half - d
                        tensor.matmul(p[:, :], lhsT=S(d), rhs=z_s[idx][:, c0:c0 + 258],
                                      start=(d == 0), stop=False)
                for d in range(1, 4):
                    for half, p in ((0, p0), (1, p1)):
                        c0 = 3 + 258 * half - d
                        ins = tensor.matmul(p[:, :], lhsT=Wr(d),
                                            rhs=z_s[(idx + 1) % NF][:, c0:c0 + 258],
                                            start=False, stop=(d == 3))
                        if d == 3:
                            ins.then_inc(sAmm, 1)

            def stage_b(i):
                # C_i[o, u] = A[128i+o, u+7] + A[(128i+o+4)%1024, u+3], u in [0,512)
                # A_s[i][:, j] holds A[128i + ., j+3]. Borrows pA slot (i+1)%2.
                tensor.wait_ge(sAcp, 2 * min(i + 2, NF))
                if i == 7:
                    tensor.wait_ge(sCcpS, 6)   # slot 0 free of B_5
                    tensor.wait_ge(sCcpV, 6)
                s = (i + 1) % 2
                bA, bB = pA[2 * s], pA[2 * s + 1]
                for mat, rhs_off in ((S(0), 4), (S(4), 0)):
                    for bank, c0 in ((bA, 0), (bB, 256)):
                        tensor.matmul(bank[:, 0:256], lhsT=mat,
                                      rhs=A_s[i][:, c0 + rhs_off:c0 + rhs_off + 256],
                                      start=(rhs_off == 4), stop=False)
                for bank, c0 in ((bA, 0), (bB, 256)):
                    ins = tensor.matmul(bank[:, 0:256], lhsT=Wr(4),
                                        rhs=A_s[(i + 1) % NF][:, c0:c0 + 256],
                                        start=False, stop=True)
                    ins.then_inc(sBmm, 1)

            for f in range(NF):
                if f == 0:
                    tensor.wait_ge(dxh, 16 * 7)
                    tensor.wait_ge(dxg, 16 * 6)
                    tensor.wait_ge(dw0, 16 * 4)
                if f >= 2:
                    tensor.wait_ge(dwq, 16 * (4 + 8 * (f // 2)))
                    tensor.wait_ge(sza, 2 * (f - 1))  # pingpong banks free
                bA, bB = mA[f % 2], mB[f % 2]
                csA, csB = slice(0, 260), slice(260, 520)
                # term order T1 (xh@wh), T3 (xh@wl), T2 (xl@wh): xl arrives
                # last on the scalar queue, so its term runs last
                for k in range(NK):
                    tensor.matmul(bA[:, :], lhsT=wh_s[:, wcol(f, k)], rhs=xh[k][:, csA],
                                  start=(k == 0), stop=False)
                    tensor.matmul(bB[:, :], lhsT=wh_s[:, wcol(f, k)], rhs=xh[k][:, csB],
                                  start=(k == 0), stop=False)
                if f <= 1:
                    tensor.wait_ge(dwq, 16 * 4)       # Q0's wl half
                for k in range(NK):
                    tensor.matmul(bA[:, :], lhsT=wl_s[:, wcol(f, k)], rhs=xh[k][:, csA],
                                  start=False, stop=False)
                    tensor.matmul(bB[:, :], lhsT=wl_s[:, wcol(f, k)], rhs=xh[k][:, csB],
                                  start=False, stop=False)
                if f == 0:
                    tensor.wait_ge(dxl, 16 * 6)
                for k in range(NK):
                    last = k == NK - 1
                    ins = tensor.matmul(bA[:, :], lhsT=wh_s[:, wcol(f, k)], rhs=xl[k][:, csA],
                                        start=False, stop=last)
                    if last:
                        ins.then_inc(smm, 1)
                    ins = tensor.matmul(bB[:, :], lhsT=wh_s[:, wcol(f, k)], rhs=xl[k][:, csB],
                                        start=False, stop=last)
                    if last:
                        ins.then_inc(smm, 1)
                # conv stages trail the main matmuls by 2 / 4 tiles
                if f >= 2:
                    stage_a(f - 2)
                if f >= 4:
                    stage_b(f - 4)
            stage_b(4)
            stage_a(6)
            stage_a(7)
            stage_b(5)
            stage_b(6)
            stage_b(7)

        @block.scalar
        def _(scalar):
            for f in (0, 1):
                for hb in range(2):
                    cs = slice(768 * f + 384 * hb, 768 * f + 384 * (hb + 1))
                    scalar.dma_start(out=wh_s[:, cs], in_=wh_in[:, cs]).then_inc(dw0, 16)
            for k in range(NK):
                scalar.dma_start(out=xl[k][:, :], in_=xl_in[128 * k:128 * (k + 1), :]).then_inc(dxl, 16)

            def c_copy_s(i):
                s = (i + 1) % 2
                scalar.wait_ge(sBmm, 2 * i + 1)
                scalar.activation(out=cc[i][:, 0:256], in_=pA[2 * s][:, 0:256],
                                  func=mybir.ActivationFunctionType.Copy).then_inc(sCcpS, 1)

            for f in range(NF):
                bias_l = bias_s[:, f:f + 1]
                for half, bank in ((0, mA[f % 2]), (1, mB[f % 2])):
                    scalar.wait_ge(smm, 2 * f + half + 1)
                    scalar.activation(out=a_s[f][:, 260 * half:260 * (half + 1)], in_=bank[:, :],
                                      bias=bias_l,
                                      func=mybir.ActivationFunctionType.Tanh).then_inc(sza, 1)
                scalar.wait_ge(sza, 2 * (f + 1))
                scalar.dma_start(out=at_out[128 * f:128 * (f + 1), :],
                                 in_=a_s[f][:, 3:515]).then_inc(dout, 16)
                if f >= 4:
                    c_copy_s(f - 4)
            for i in range(4, NF):
                c_copy_s(i)

        @block.gpsimd
        def _(gpsimd):
            for k in range(1, NK, 2):
                for ph in range(2):
                    rs = slice(128 * k + 64 * ph, 128 * k + 64 * (ph + 1))
                    gpsimd.dma_start(out=xh[k][64 * ph:64 * (ph + 1), :],
                                     in_=xh_in[rs, :]).then_inc(dxg, 16)
            for j in range(3):
                gpsimd.dma_start(out=sh[:, 384 * j:384 * (j + 1)],
                                 in_=sh_in[:, 384 * j:384 * (j + 1)]).then_inc(dsh, 16)
            for f in range(NF):
                gpsimd.wait_ge(szz, f + 1)
                gpsimd.dma_start(out=zt_out[128 * f:128 * (f + 1), :],
                                 in_=z_s[f][:, 3:515]).then_inc(dzout, 16)

        @block.vector
        def _(vector):
            def copy_a(idx):
                for half in range(2):
                    vector.wait_ge(sAmm, 2 * idx + half + 1)
                    vector.tensor_scalar_add(out=A_s[idx][:, 258 * half:258 * (half + 1)],
                                             in0=pA[2 * (idx % 2) + half][:, :],
                                             scalar1=0.0).then_inc(sAcp, 1)

            def c_copy_v(i):
                s = (i + 1) % 2
                vector.wait_ge(sBmm, 2 * i + 2)
                vector.tensor_scalar_add(out=cc[i][:, 256:512], in0=pA[2 * s + 1][:, 0:256],
                                         scalar1=0.0).then_inc(sCcpV, 1)

            for f in range(NF):
                vector.wait_ge(sza, 2 * (f + 1))
                vector.tensor_scalar(out=z_s[f][:, :], in0=a_s[f][:, :], scalar1=0.0,
                                     scalar2=None, op0=mybir.AluOpType.is_gt).then_inc(szz, 1)
                if f >= 2:
                    copy_a(f - 2)
                if f >= 4:
                    c_copy_v(f - 4)
            copy_a(6)
            c_copy_v(4)
            copy_a(7)
            c_copy_v(5)
            c_copy_v(6)
            c_copy_v(7)

    return nc


_NC = None


def _prep_weights(W, b):
    wh4 = W.astype(bf16_np)
    wl4 = (W - wh4.astype(np.float32)).astype(bf16_np)

    # pack [768, 1024] -> [128, (f, k, o)]: col = 768f + 128k + o
    def pack(w):
        return np.ascontiguousarray(
            w.reshape(NK, 128, NF, 128).transpose(1, 2, 0, 3).reshape(128, NF * NK * 128))
    biasv = np.ascontiguousarray(b.reshape(NF, 128).T.astype(np.float32))  # [128, 8]

    # S_d[p, o] = 1 iff p == o + d; W_d[p, o] = 1 iff p == o + d - 128
    s = np.zeros((128, 9 * 128), dtype=bf16_np)
    for d in range(5):
        for o in range(0, 128 - d):
            s[o + d, 128 * d + o] = 1
    for d in range(1, 5):
        for o in range(128 - d, 128):
            s[o + d - 128, 128 * (4 + d) + o] = 1
    return pack(wh4), pack(wl4), biasv, s


def kernel(x: np.ndarray, W: np.ndarray, b: np.ndarray):
    global _NC, LAST_RESULTS
    x = np.asarray(x, dtype=np.float32)
    W = np.asarray(W, dtype=np.float32)
    b = np.asarray(b, dtype=np.float32)

    if _NC is None:
        _NC = _build_module()
    nc = _NC

    wh_p, wl_p, biasv, shifts = _prep_weights(W, b)

    in_maps = []
    metas = []
    for c in range(8):
        bi, h = c // 2, c % 2
        t0 = OWN * h
        lo, hi = t0 - 3, t0 + 517  # 520 rows
        xc = np.zeros((T, IN_DIM), dtype=np.float32)
        src_lo, src_hi = max(lo, 0), min(hi, SEQ)
        xc[src_lo - lo:src_hi - lo, :] = x[bi, src_lo:src_hi, :]
        xt = np.ascontiguousarray(xc.T)              # [768, 520] f32
        xhc = xt.astype(bf16_np)
        xlc = (xt - xhc.astype(np.float32)).astype(bf16_np)
        in_maps.append({"xh": xhc, "xl": xlc, "wh": wh_p, "wl": wl_p,
                        "biasv": biasv, "sh": shifts})
        metas.append((bi, t0))

    res = run_bass_kernel_spmd(nc, in_maps, list(range(8)))
    LAST_RESULTS = res

    a_full = np.empty((BATCH, SEQ, F_DIM), dtype=np.float32)
    z_full = np.empty((BATCH, SEQ, F_DIM), dtype=np.float32)
    zc_full = np.empty((BATCH, SEQ, F_DIM), dtype=np.float32)
    for c in range(8):
        bi, t0 = metas[c]
        r = res.results[c]
        a_full[bi, t0:t0 + OWN, :] = r["at"].astype(np.float32).T
        z_full[bi, t0:t0 + OWN, :] = r["zt"].astype(np.float32).T
        zc_full[bi, t0:t0 + OWN, :] = r["ct"].astype(np.float32).T

    # Boundary fixup: device z at t<0 / t>=SEQ is (tanh(b) > 0) instead of the
    # conv's zero padding, affecting z_conv at t in [0,3) and [SEQ-4, SEQ).
    # Recompute those columns from the exact device z (no-op when b == 0).
    for t in list(range(3)) + list(range(SEQ - 4, SEQ)):
        acc = np.zeros((BATCH, F_DIM), dtype=np.float32)
        for d in range(8):
            ts = t + 4 - d
            if 0 <= ts < SEQ:
                acc += np.roll(z_full[:, ts, :], -d, axis=-1)
        zc_full[:, t, :] = acc
    return (a_full, z_full, zc_full)
